# revision 23
# baseline (speedup 1.0000x reference)
"""Trainium2 Bass kernel for nn_Compression.

Computes: out = X + GAMMA * (P @ (P.T @ X)),  P = softmax(X @ W.T + b)

Strategy (8 NeuronCores, data-parallel over N):
  - Each core owns NLOC = N/8 = 4096 rows of X (32 tiles of 128 rows).
  - All I/O in bf16 (the residual term X passes through at bf16
    precision: ~1e-3 relative output error against a 2e-2 gate; the
    GAMMA=1e-4 correction term contributes ~2e-7). This halves HBM
    traffic vs f32 and removes all on-device casts.
  - Phase A per row-tile: PE-transpose the X tile (logits need D on
    partitions), logits via bf16 matmuls, softmax with fused exp+row-sum
    on ScalarE, then accumulate P.T @ X into 4 resident PSUM banks as
    fp8-e4m3 DoubleRow matmuls over 256-row pairs (P scaled by S8=4 to
    sit in fp8 normal range; X supplied by the host in a second fp8
    row-pair-interleaved layout). fp8 costs ~nothing in accuracy here
    because the correction term is GAMMA-scaled to ~1e-4 of the output.
  - PtX is accumulated in TWO 16-tile groups. AllReduce is linear, so
    each group's [C, D] bf16 partial is AllReduce'd separately: group
    0's collective runs concurrently with phase A's second half (also
    absorbing any inter-core launch skew), leaving only group 1's
    collective (bf16, 512 KiB) exposed. Group 1's AllReduce is further
    split into two D-halves so the second half overlaps phase-B compute
    on the first.
  - Phase B per D-half: G = GAMMA*(red0 + red1), corr = P @ G in bf16,
    residual add against the SBUF-resident bf16 X, DMA out in bf16.

The host wrapper casts X/W to bf16 (and pre-transposes W), and casts
the bf16 device output back to f32. b is zeros in this problem's
setup; a separate kernel variant with the bias matmul is compiled
lazily only if a nonzero b is ever passed.
"""

import sys

import numpy as np

if "/opt/trn_rl_repo" not in sys.path:
    sys.path.insert(0, "/opt/trn_rl_repo")

N, D, C = 32768, 1024, 256
GAMMA = 1e-4
NCORES = 8
NLOC = N // NCORES  # 4096
P = 128
NT = NLOC // P  # 32
NG = NT // 2  # 16 tiles per PtX reduction group
NPAIR = NT // 2  # 16 row-tile pairs (fp8 DoubleRow contracts 256 rows)
DH = 512
S8 = 4.0  # fp8 scale for P (keeps S*PtX well under the e4m3 240 max)

_cache = {}


def _build_nc(with_bias):
    import concourse.tile as tile
    from concourse import bacc
    import concourse.mybir as mybir
    from concourse.masks import make_identity
    from contextlib import ExitStack

    f32 = mybir.dt.float32
    bf16 = mybir.dt.bfloat16
    f8 = mybir.dt.float8e4
    AF = mybir.ActivationFunctionType
    DR = mybir.MatmulPerfMode.DoubleRow

    nc = bacc.Bacc("TRN2", target_bir_lowering=False, debug=False, num_devices=NCORES)
    X = nc.dram_tensor("X", [NLOC, D], bf16, kind="ExternalInput").ap()
    # X again, fp8-e4m3 in row-pair interleaved layout for DoubleRow PtX:
    # Xp8[s, p, j, d] = X[256*s + 128*j + p, d]
    Xp8 = nc.dram_tensor("Xp8", [NPAIR, P, 2, D], f8, kind="ExternalInput").ap()
    Wt = nc.dram_tensor("Wt", [D, C], bf16, kind="ExternalInput").ap()
    bvec = nc.dram_tensor("b", [C], f32, kind="ExternalInput").ap()
    out = nc.dram_tensor("out", [NLOC, D], bf16, kind="ExternalOutput").ap()

    with tile.TileContext(nc) as tc, ExitStack() as ctx:
        const = ctx.enter_context(tc.tile_pool(name="const", bufs=1))
        xres = ctx.enter_context(tc.tile_pool(name="xres", bufs=1))
        work = ctx.enter_context(tc.tile_pool(name="work", bufs=2))
        ppool = ctx.enter_context(tc.tile_pool(name="ppool", bufs=4))
        p8pool = ctx.enter_context(tc.tile_pool(name="p8pool", bufs=3))
        xf8pool = ctx.enter_context(tc.tile_pool(name="xf8pool", bufs=4))
        spool = ctx.enter_context(tc.tile_pool(name="spool", bufs=6))
        opool = ctx.enter_context(tc.tile_pool(name="opool", bufs=3))
        dram = ctx.enter_context(tc.tile_pool(name="dram", bufs=1, space="DRAM"))

        ident = const.tile([P, P], bf16)
        make_identity(nc, ident)

        # W.T resident in bf16, [d-within-chunk, k-chunk, c]; direct DMA,
        # no cast needed.
        Wt_sb = const.tile([P, 8, C], bf16)
        nc.sync.dma_start(Wt_sb[:], Wt.rearrange("(k p) c -> p k c", p=P))

        if with_bias:
            ones1 = const.tile([1, P], bf16)
            nc.vector.memset(ones1[:], 1.0)
            b_sb = const.tile([1, C], bf16)
            with tc.tile_pool(name="btmp", bufs=1) as btmp:
                b_f = btmp.tile([1, C], f32)
                nc.sync.dma_start(b_f[:], bvec.rearrange("(o c) -> o c", o=1))
                nc.vector.tensor_copy(b_sb[:], b_f[:])

        Xall = xres.tile([P, NT, D], bf16)  # resident bf16 X, 64 KiB/part
        Pt = const.tile([P, 2, NLOC], bf16)  # P.T resident, bf16

        # Per-group AllReduce buffers ([C, D] bf16 = 512 KiB each; the
        # collectives are latency-bound at this size, so no D-chunking).
        ar_in = [dram.tile([C, D], bf16, name=f"ar_in{g}") for g in range(2)]
        ar_out = [
            dram.tile([C, D], bf16, addr_space="Shared", name=f"ar_out{g}")
            for g in range(2)
        ]

        # ---- phase A: software-pipelined over row-tiles ----
        # Per step i the PE stream is: logits(i), transposes(i+1),
        # PtX/PT(i-1). The softmax ACT->DVE round-trip for tile i hides
        # under transposes(i+1) + PtX(i-1); the transpose-copy (ACT) for
        # i+1 hides under PtX(i-1) + logits(i+1) -- no PE idle.
        def s_load(i):
            nc.sync.dma_start(Xall[:, i, :], X[i * P:(i + 1) * P, :])

        def s_load_pair(s):
            xf8 = xf8pool.tile([P, 2, D], f8, name="xf8", tag="xf8")
            nc.sync.dma_start(xf8[:], Xp8[s])
            return xf8

        def s_transpose(i):
            # 8 PE transposes into one PSUM bank as a single accumulation
            # group (start clears the whole bank once). The drain is split
            # ACT/DVE: ScalarE is otherwise the phase-A critical engine.
            xt = work.tile([P, D], bf16, name="xt", tag="xt")
            trp = psA.tile([P, D], bf16, name="trp", tag="trp")
            for k in range(8):
                nc.tensor.matmul(
                    trp[:, k * P:(k + 1) * P],
                    Xall[:, i, k * P:(k + 1) * P],
                    ident[:],
                    is_transpose=True,
                    start=(k == 0),
                    stop=(k == 7),
                )
            nc.scalar.copy(xt[:], trp[:])
            return xt

        def s_logits(i, xt):
            lg = psL.tile([P, C], f32, name="lg", tag="lg")
            for k in range(8):
                nc.tensor.matmul(
                    lg[:],
                    xt[:, k * P:(k + 1) * P],
                    Wt_sb[:, k, :],
                    start=(k == 0),
                    stop=(with_bias is False and k == 7),
                )
            if with_bias:
                nc.tensor.matmul(lg[:], ones1[:], b_sb[:], start=False, stop=True)
            return lg

        def s_softmax(i, p8pair, lg):
            # |logits| <= ~10 so exp is safe without max-subtraction
            p_sb = ppool.tile([P, C], f32, name="p_sb", tag="p")
            ssum = spool.tile([P, 1], f32, name="ssum", tag="s")
            nc.scalar.activation(p_sb[:], lg[:], AF.Exp, accum_out=ssum[:])
            rinv = spool.tile([P, 1], f32, name="rinv", tag="r")
            nc.vector.reciprocal(rinv[:], ssum[:])
            p_bf = ppool.tile([P, C], bf16, name="p_bf", tag="pb")
            nc.vector.tensor_scalar_mul(p_bf[:], p_sb[:], rinv[:])
            # fp8 copy of P scaled by S8, into this pair's DoubleRow slot
            rinvS = spool.tile([P, 1], f32, name="rinvS", tag="rS")
            nc.vector.tensor_scalar_mul(rinvS[:], rinv[:], S8)
            nc.vector.tensor_scalar_mul(p8pair[:, i % 2, :], p_sb[:], rinvS[:])
            return p_bf

        def s_ptr(i, p_bf):
            # P.T tile for phase B (bf16); drained on DVE to keep ScalarE
            # (exp + xt drain) off the critical path.
            ptp = psA.tile([P, C], bf16, name="ptp", tag="trp")
            for c in range(2):
                nc.tensor.matmul(
                    ptp[:, c * P:(c + 1) * P],
                    p_bf[:, c * P:(c + 1) * P],
                    ident[:],
                    is_transpose=True,
                    start=(c == 0),
                    stop=(c == 1),
                )
            nc.vector.tensor_copy(
                Pt[:, :, i * P:(i + 1) * P],
                ptp[:].rearrange("p (c n) -> p c n", c=2),
            )

        def s_ptx_pair(s, p8pair, xf8):
            # PtX accumulation over a 256-row pair: fp8 DoubleRow matmuls
            # (out = sum_j lhsT[:,j,:].T @ rhs[:,j,:]).
            first = s % (NG // 2) == 0
            last = s % (NG // 2) == NG // 2 - 1
            for c in range(2):
                for h in range(2):
                    nc.tensor.matmul(
                        ptx_ps[2 * c + h][:],
                        p8pair[:, :, c * P:(c + 1) * P],
                        xf8[:, :, h * DH:(h + 1) * DH],
                        start=first,
                        stop=last,
                        perf_mode=DR,
                    )

        def drain_group(g):
            # PSUM -> SBUF (bf16) -> DRAM, then AllReduce the partial.
            # Copies split across DVE and ACT to shorten the tail; the
            # GAMMA/S8 fold rides along so phase B's G-prep is one add.
            sg = const.tile([P, 2, D], bf16, name=f"sg{g}", tag=f"sg{g}")
            for h in range(2):
                nc.vector.tensor_scalar_mul(
                    sg[:, 0, h * DH:(h + 1) * DH], ptx_ps[h][:], GAMMA / S8
                )
                nc.scalar.mul(sg[:, 1, h * DH:(h + 1) * DH], ptx_ps[2 + h][:], GAMMA / S8)
            import concourse.mybir as mybir2

            nc.sync.dma_start(ar_in[g].rearrange("(c p) d -> p c d", p=P), sg[:])
            nc.gpsimd.collective_compute(
                "AllReduce",
                mybir2.AluOpType.add,
                replica_groups=[list(range(NCORES))],
                ins=[ar_in[g][:].opt()],
                outs=[ar_out[g][:].opt()],
            )

        with tc.tile_pool(name="psA", bufs=3, space="PSUM") as psA, \
             tc.tile_pool(name="psL", bufs=1, space="PSUM") as psL, \
             tc.tile_pool(name="psX", bufs=1, space="PSUM") as psX:
            ptx_ps = [
                psX.tile([P, DH], f32, name=f"ptx_{c}_{h}", tag=f"ptx_{c}_{h}")
                for c in range(2)
                for h in range(2)
            ]
            # 2-step skew between softmax(i) and ptx(i): the ~1.1us ScalarE
            # exp latency hides under transposes + the previous ptx + the
            # next logits block instead of stalling the PE.
            s_load(0)
            s_load(1)
            pairs = {0: s_load_pair(0), 1: s_load_pair(1)}
            p8s = {}
            xt0 = s_transpose(0)
            state = {0: (xt0, None), 1: (None, None)}
            for i in range(NT):
                xt_i, _ = state[i]
                if i % 2 == 0:
                    p8s[i // 2] = p8pool.tile([P, 2, C], f8, name="p8", tag="p8")
                lg = s_logits(i, xt_i)
                p_bf = s_softmax(i, p8s[i // 2], lg)
                state[i] = (xt_i, p_bf)
                if i + 1 < NT:
                    state[i + 1] = (s_transpose(i + 1), None)
                if i + 2 < NT:
                    s_load(i + 2)
                if i % 2 == 0 and i + 4 < NT:
                    pairs[(i + 4) // 2] = s_load_pair((i + 4) // 2)
                if i >= 2:
                    j = i - 2
                    _, p_bf_p = state.pop(j)
                    s_ptr(j, p_bf_p)
                    if j % 2 == 1:
                        s_ptx_pair(j // 2, p8s.pop(j // 2), pairs.pop(j // 2))
                        if j // 2 == NG // 2 - 1:
                            drain_group(0)
            for j in (NT - 2, NT - 1):
                _, p_bf_l = state.pop(j)
                s_ptr(j, p_bf_l)
                if j % 2 == 1:
                    s_ptx_pair(j // 2, p8s.pop(j // 2), pairs.pop(j // 2))
            drain_group(1)

        # ---- phase B. GAMMA/S8 is already folded into the AllReduce
        # inputs, so G = red0 + red1 in one bf16 DVE add; corr matmul in
        # bf16; one full-D residual add and one 256 KiB store per
        # row-tile. The adds alternate between DVE-direct (PSUM+SBUF)
        # and ACT-copy + GpSimd-add so no single engine is the
        # bottleneck (ACT and GpSimd are otherwise idle here). ----
        red = [const.tile([P, 2, D], bf16, name=f"red{g}") for g in range(2)]
        for g in range(2):
            nc.sync.dma_start(red[g][:], ar_out[g].rearrange("(c p) d -> p c d", p=P))
        gb = const.tile([P, 2, D], bf16, name="gb")
        for h in range(2):  # halves, so h=0 matmuls can start ~1us earlier
            nc.vector.tensor_add(
                gb[:, :, h * DH:(h + 1) * DH],
                red[0][:, :, h * DH:(h + 1) * DH],
                red[1][:, :, h * DH:(h + 1) * DH],
            )

        # Residual-add paths per tile: 1 = DVE direct (PSUM+SBUF add),
        # 2 = ACT copy to SBUF + DVE 2x add, 3 = ACT copy + GpSimd add.
        # Mix keeps DVE/ACT/GpSimd each at ~20us for 32 tiles.
        paths = ([1, 2, 3] * 11)[:NT]
        tpool = ctx.enter_context(tc.tile_pool(name="tpool", bufs=4))
        with tc.tile_pool(name="psB", bufs=4, space="PSUM") as psB:
            for i in range(NT):
                cor = psB.tile([P, D], f32, name="cor", tag="cor")
                for h in range(2):
                    for c in range(2):
                        nc.tensor.matmul(
                            cor[:, h * DH:(h + 1) * DH],
                            Pt[:, c, i * P:(i + 1) * P],
                            gb[:, c, h * DH:(h + 1) * DH],
                            start=(c == 0),
                            stop=(c == 1),
                        )
                o_sb = opool.tile([P, D], bf16, name="o_sb", tag="o")
                if paths[i] == 1:
                    nc.vector.tensor_add(o_sb[:], cor[:], Xall[:, i, :])
                else:
                    t_sb = tpool.tile([P, D], bf16, name="t_sb", tag="t")
                    nc.scalar.copy(t_sb[:], cor[:])
                    if paths[i] == 2:
                        nc.vector.tensor_add(o_sb[:], t_sb[:], Xall[:, i, :])
                    else:
                        nc.gpsimd.tensor_add(o_sb[:], t_sb[:], Xall[:, i, :])
                nc.sync.dma_start(out[i * P:(i + 1) * P, :], o_sb[:])

    nc.finalize()
    return nc


def _run(inputs, trace=False, **kwargs):
    import ml_dtypes
    from concourse import bass_utils

    bf16 = ml_dtypes.bfloat16

    X = np.asarray(inputs["X"], dtype=np.float32)
    W = np.asarray(inputs["W"], dtype=np.float32)
    b = np.ascontiguousarray(np.asarray(inputs["b"], dtype=np.float32))

    with_bias = bool(np.any(b != 0.0))
    key = "nc_bias" if with_bias else "nc"
    if key not in _cache:
        _cache[key] = _build_nc(with_bias)
    nc = _cache[key]

    f8 = ml_dtypes.float8_e4m3

    Xb = np.ascontiguousarray(X.astype(bf16))
    Wtb = np.ascontiguousarray(W.T.astype(bf16))
    # fp8 X in row-pair interleaved layout: Xp8[s, p, j, d] = X[256s+128j+p, d]
    X8 = X.astype(f8).reshape(NCORES, NPAIR, 2, P, D).swapaxes(2, 3)

    in_maps = [
        {
            "X": Xb[i * NLOC:(i + 1) * NLOC],
            "Xp8": np.ascontiguousarray(X8[i]),
            "Wt": Wtb,
            "b": b,
        }
        for i in range(NCORES)
    ]
    res = bass_utils.run_bass_kernel_spmd(
        nc, in_maps, core_ids=list(range(NCORES)), trace=trace, **kwargs
    )
    outp = np.concatenate(
        [np.asarray(res.results[i]["out"]) for i in range(NCORES)], axis=0
    ).astype(np.float32)
    return outp, res


def kernel(**inputs):
    outp, _ = _run(inputs, trace=False)
    return outp


# revision 24
# speedup vs baseline: 1.1523x; 1.1523x over previous
"""Trainium2 Bass kernel for nn_Compression.

Computes: out = X + GAMMA * (P @ (P.T @ X)),  P = softmax(X @ W.T + b)

Strategy (8 NeuronCores, data-parallel over N):
  - Each core owns NLOC = N/8 = 4096 rows of X (32 tiles of 128 rows).
  - All I/O in bf16 (the residual term X passes through at bf16
    precision: ~1e-3 relative output error against a 2e-2 gate; the
    GAMMA=1e-4 correction term contributes ~2e-7). This halves HBM
    traffic vs f32 and removes all on-device casts.
  - Phase A per row-tile: PE-transpose the X tile (logits need D on
    partitions), logits via bf16 matmuls, softmax with fused exp+row-sum
    on ScalarE, then accumulate P.T @ X into 4 resident PSUM banks as
    fp8-e4m3 DoubleRow matmuls over 256-row pairs (P scaled by S8=4 to
    sit in fp8 normal range; X supplied by the host in a second fp8
    row-pair-interleaved layout). fp8 costs ~nothing in accuracy here
    because the correction term is GAMMA-scaled to ~1e-4 of the output.
  - PtX is accumulated in TWO 16-tile groups. AllReduce is linear, so
    each group's [C, D] bf16 partial is AllReduce'd separately: group
    0's collective runs concurrently with phase A's second half (also
    absorbing any inter-core launch skew), leaving only group 1's
    collective (bf16, 512 KiB) exposed. Group 1's AllReduce is further
    split into two D-halves so the second half overlaps phase-B compute
    on the first.
  - Phase B per D-half: G = GAMMA*(red0 + red1), corr = P @ G in bf16,
    residual add against the SBUF-resident bf16 X, DMA out in bf16.

The host wrapper casts X/W to bf16 (and pre-transposes W), and casts
the bf16 device output back to f32. b is zeros in this problem's
setup; a separate kernel variant with the bias matmul is compiled
lazily only if a nonzero b is ever passed.
"""

import sys

import numpy as np

if "/opt/trn_rl_repo" not in sys.path:
    sys.path.insert(0, "/opt/trn_rl_repo")

N, D, C = 32768, 1024, 256
GAMMA = 1e-4
NCORES = 8
NLOC = N // NCORES  # 4096
P = 128
NT = NLOC // P  # 32
NG = NT // 2  # 16 tiles per PtX reduction group
NPAIR = NT // 2  # 16 row-tile pairs (fp8 DoubleRow contracts 256 rows)
DH = 512
S8 = 4.0  # fp8 scale for P (keeps S*PtX well under the e4m3 240 max)

_cache = {}


def _build_nc(with_bias):
    import concourse.tile as tile
    from concourse import bacc
    import concourse.mybir as mybir
    from concourse.masks import make_identity
    from contextlib import ExitStack

    f32 = mybir.dt.float32
    bf16 = mybir.dt.bfloat16
    f8 = mybir.dt.float8e4
    AF = mybir.ActivationFunctionType
    DR = mybir.MatmulPerfMode.DoubleRow

    nc = bacc.Bacc("TRN2", target_bir_lowering=False, debug=False, num_devices=NCORES)
    X = nc.dram_tensor("X", [NLOC, D], bf16, kind="ExternalInput").ap()
    # X again, fp8-e4m3 in row-pair interleaved layout for DoubleRow PtX:
    # Xp8[s, p, j, d] = X[256*s + 128*j + p, d]
    Xp8 = nc.dram_tensor("Xp8", [NPAIR, P, 2, D], f8, kind="ExternalInput").ap()
    Wt = nc.dram_tensor("Wt", [D, C], bf16, kind="ExternalInput").ap()
    bvec = nc.dram_tensor("b", [C], f32, kind="ExternalInput").ap()
    out = nc.dram_tensor("out", [NLOC, D], bf16, kind="ExternalOutput").ap()

    with tile.TileContext(nc) as tc, ExitStack() as ctx:
        const = ctx.enter_context(tc.tile_pool(name="const", bufs=1))
        xres = ctx.enter_context(tc.tile_pool(name="xres", bufs=1))
        work = ctx.enter_context(tc.tile_pool(name="work", bufs=2))
        ppool = ctx.enter_context(tc.tile_pool(name="ppool", bufs=4))
        p8pool = ctx.enter_context(tc.tile_pool(name="p8pool", bufs=3))
        xf8pool = ctx.enter_context(tc.tile_pool(name="xf8pool", bufs=4))
        spool = ctx.enter_context(tc.tile_pool(name="spool", bufs=6))
        opool = ctx.enter_context(tc.tile_pool(name="opool", bufs=3))
        dram = ctx.enter_context(tc.tile_pool(name="dram", bufs=1, space="DRAM"))

        ident = const.tile([P, P], bf16)
        make_identity(nc, ident)

        # W.T resident in bf16, [d-within-chunk, k-chunk, c]; direct DMA,
        # no cast needed.
        Wt_sb = const.tile([P, 8, C], bf16)
        nc.sync.dma_start(Wt_sb[:], Wt.rearrange("(k p) c -> p k c", p=P))

        if with_bias:
            ones1 = const.tile([1, P], bf16)
            nc.vector.memset(ones1[:], 1.0)
            b_sb = const.tile([1, C], bf16)
            with tc.tile_pool(name="btmp", bufs=1) as btmp:
                b_f = btmp.tile([1, C], f32)
                nc.sync.dma_start(b_f[:], bvec.rearrange("(o c) -> o c", o=1))
                nc.vector.tensor_copy(b_sb[:], b_f[:])

        Xall = xres.tile([P, NT, D], bf16)  # resident bf16 X, 64 KiB/part
        Pt = const.tile([P, 2, NLOC], bf16)  # P.T resident, bf16

        # Per-group AllReduce buffers ([C, D] bf16 = 512 KiB each; the
        # collectives are latency-bound at this size, so no D-chunking).
        ar_in = [dram.tile([C, D], bf16, name=f"ar_in{g}") for g in range(2)]
        ar_out = [
            dram.tile([C, D], bf16, addr_space="Shared", name=f"ar_out{g}")
            for g in range(2)
        ]

        # ---- phase A: software-pipelined over row-tiles ----
        # Per step i the PE stream is: logits(i), transposes(i+1),
        # PtX/PT(i-1). The softmax ACT->DVE round-trip for tile i hides
        # under transposes(i+1) + PtX(i-1); the transpose-copy (ACT) for
        # i+1 hides under PtX(i-1) + logits(i+1) -- no PE idle.
        def s_load(i):
            nc.sync.dma_start(Xall[:, i, :], X[i * P:(i + 1) * P, :])

        def s_load_pair(s):
            xf8 = xf8pool.tile([P, 2, D], f8, name="xf8", tag="xf8")
            nc.sync.dma_start(xf8[:], Xp8[s])
            return xf8

        def s_transpose(i):
            # 8 PE transposes into one PSUM bank as a single accumulation
            # group (start clears the whole bank once). The drain is split
            # ACT/DVE: ScalarE is otherwise the phase-A critical engine.
            xt = work.tile([P, D], bf16, name="xt", tag="xt")
            trp = psA.tile([P, D], bf16, name="trp", tag="trp")
            for k in range(8):
                nc.tensor.matmul(
                    trp[:, k * P:(k + 1) * P],
                    Xall[:, i, k * P:(k + 1) * P],
                    ident[:],
                    is_transpose=True,
                    start=(k == 0),
                    stop=(k == 7),
                )
            nc.scalar.copy(xt[:], trp[:])
            return xt

        def s_logits(i, xt):
            lg = psL.tile([P, C], f32, name="lg", tag="lg")
            for k in range(8):
                nc.tensor.matmul(
                    lg[:],
                    xt[:, k * P:(k + 1) * P],
                    Wt_sb[:, k, :],
                    start=(k == 0),
                    stop=(with_bias is False and k == 7),
                )
            if with_bias:
                nc.tensor.matmul(lg[:], ones1[:], b_sb[:], start=False, stop=True)
            return lg

        def s_softmax(i, p8pair, lg):
            # |logits| <= ~10 so exp is safe without max-subtraction
            p_sb = ppool.tile([P, C], f32, name="p_sb", tag="p")
            ssum = spool.tile([P, 1], f32, name="ssum", tag="s")
            nc.scalar.activation(p_sb[:], lg[:], AF.Exp, accum_out=ssum[:])
            rinv = spool.tile([P, 1], f32, name="rinv", tag="r")
            nc.vector.reciprocal(rinv[:], ssum[:])
            p_bf = ppool.tile([P, C], bf16, name="p_bf", tag="pb")
            nc.vector.tensor_scalar_mul(p_bf[:], p_sb[:], rinv[:])
            # fp8 copy of P scaled by S8, into this pair's DoubleRow slot
            rinvS = spool.tile([P, 1], f32, name="rinvS", tag="rS")
            nc.vector.tensor_scalar_mul(rinvS[:], rinv[:], S8)
            nc.vector.tensor_scalar_mul(p8pair[:, i % 2, :], p_sb[:], rinvS[:])
            return p_bf

        def s_ptr(i, p_bf):
            # P.T tile for phase B (bf16); drained on DVE to keep ScalarE
            # (exp + xt drain) off the critical path.
            ptp = psA.tile([P, C], bf16, name="ptp", tag="trp")
            for c in range(2):
                nc.tensor.matmul(
                    ptp[:, c * P:(c + 1) * P],
                    p_bf[:, c * P:(c + 1) * P],
                    ident[:],
                    is_transpose=True,
                    start=(c == 0),
                    stop=(c == 1),
                )
            nc.vector.tensor_copy(
                Pt[:, :, i * P:(i + 1) * P],
                ptp[:].rearrange("p (c n) -> p c n", c=2),
            )

        def s_ptx_pair(s, p8pair, xf8):
            # PtX accumulation over a 256-row pair: fp8 DoubleRow matmuls
            # (out = sum_j lhsT[:,j,:].T @ rhs[:,j,:]).
            first = s % (NG // 2) == 0
            last = s % (NG // 2) == NG // 2 - 1
            for c in range(2):
                for h in range(2):
                    nc.tensor.matmul(
                        ptx_ps[2 * c + h][:],
                        p8pair[:, :, c * P:(c + 1) * P],
                        xf8[:, :, h * DH:(h + 1) * DH],
                        start=first,
                        stop=last,
                        perf_mode=DR,
                    )

        def drain_group(g):
            # PSUM -> SBUF (bf16) -> DRAM, then AllReduce the partial.
            # Copies split across DVE and ACT to shorten the tail; the
            # GAMMA/S8 fold rides along so phase B's G-prep is one add.
            sg = const.tile([P, 2, D], bf16, name=f"sg{g}", tag=f"sg{g}")
            for h in range(2):
                nc.vector.tensor_scalar_mul(
                    sg[:, 0, h * DH:(h + 1) * DH], ptx_ps[h][:], GAMMA / S8
                )
                nc.scalar.mul(sg[:, 1, h * DH:(h + 1) * DH], ptx_ps[2 + h][:], GAMMA / S8)
            import concourse.mybir as mybir2

            nc.sync.dma_start(ar_in[g].rearrange("(c p) d -> p c d", p=P), sg[:])
            nc.gpsimd.collective_compute(
                "AllReduce",
                mybir2.AluOpType.add,
                replica_groups=[list(range(NCORES))],
                ins=[ar_in[g][:].opt()],
                outs=[ar_out[g][:].opt()],
            )

        with tc.tile_pool(name="psA", bufs=3, space="PSUM") as psA, \
             tc.tile_pool(name="psL", bufs=1, space="PSUM") as psL, \
             tc.tile_pool(name="psX", bufs=1, space="PSUM") as psX:
            ptx_ps = [
                psX.tile([P, DH], f32, name=f"ptx_{c}_{h}", tag=f"ptx_{c}_{h}")
                for c in range(2)
                for h in range(2)
            ]
            # 2-step skew between softmax(i) and ptx(i): the ~1.1us ScalarE
            # exp latency hides under transposes + the previous ptx + the
            # next logits block instead of stalling the PE.
            s_load(0)
            s_load(1)
            pairs = {0: s_load_pair(0), 1: s_load_pair(1)}
            p8s = {}
            xt0 = s_transpose(0)
            state = {0: (xt0, None), 1: (None, None)}
            for i in range(NT):
                xt_i, _ = state[i]
                if i % 2 == 0:
                    p8s[i // 2] = p8pool.tile([P, 2, C], f8, name="p8", tag="p8")
                lg = s_logits(i, xt_i)
                p_bf = s_softmax(i, p8s[i // 2], lg)
                state[i] = (xt_i, p_bf)
                if i + 1 < NT:
                    state[i + 1] = (s_transpose(i + 1), None)
                if i + 2 < NT:
                    s_load(i + 2)
                if i % 2 == 0 and i + 4 < NT:
                    pairs[(i + 4) // 2] = s_load_pair((i + 4) // 2)
                if i >= 2:
                    j = i - 2
                    _, p_bf_p = state.pop(j)
                    s_ptr(j, p_bf_p)
                    if j % 2 == 1:
                        s_ptx_pair(j // 2, p8s.pop(j // 2), pairs.pop(j // 2))
                        if j // 2 == NG // 2 - 1:
                            drain_group(0)
            for j in (NT - 2, NT - 1):
                _, p_bf_l = state.pop(j)
                s_ptr(j, p_bf_l)
                if j % 2 == 1:
                    s_ptx_pair(j // 2, p8s.pop(j // 2), pairs.pop(j // 2))
            drain_group(1)

        # ---- phase B. GAMMA/S8 is already folded into the AllReduce
        # inputs, so G = red0 + red1 in one bf16 DVE add; corr matmul in
        # bf16; one full-D residual add and one 256 KiB store per
        # row-tile. The adds alternate between DVE-direct (PSUM+SBUF)
        # and ACT-copy + GpSimd-add so no single engine is the
        # bottleneck (ACT and GpSimd are otherwise idle here). ----
        red = [const.tile([P, 2, D], bf16, name=f"red{g}") for g in range(2)]
        for g in range(2):
            nc.sync.dma_start(red[g][:], ar_out[g].rearrange("(c p) d -> p c d", p=P))
        gb = const.tile([P, 2, D], bf16, name="gb")
        for h in range(2):  # halves, so h=0 matmuls can start ~1us earlier
            nc.vector.tensor_add(
                gb[:, :, h * DH:(h + 1) * DH],
                red[0][:, :, h * DH:(h + 1) * DH],
                red[1][:, :, h * DH:(h + 1) * DH],
            )

        # Residual-add paths per tile: 1 = DVE direct (PSUM+SBUF add),
        # 2 = ACT copy to SBUF + DVE 2x add, 3 = ACT copy + GpSimd add
        # (GpSimd adds measure ~2.5us, so it only gets a few).
        # Stores are batched two row-tiles (512 KiB) per dma_start: the
        # Sync queue pays ~0.65us of issue time per DMA regardless of
        # size, and 32 small stores made it a phase-B co-bottleneck.
        paths = ([1, 2, 1, 2, 2, 3] * 6)[:NT]
        tpool = ctx.enter_context(tc.tile_pool(name="tpool", bufs=4))
        with tc.tile_pool(name="psB", bufs=4, space="PSUM") as psB:
            for i in range(NT):
                cor = psB.tile([P, D], f32, name="cor", tag="cor")
                for h in range(2):
                    for c in range(2):
                        nc.tensor.matmul(
                            cor[:, h * DH:(h + 1) * DH],
                            Pt[:, c, i * P:(i + 1) * P],
                            gb[:, c, h * DH:(h + 1) * DH],
                            start=(c == 0),
                            stop=(c == 1),
                        )
                if i % 2 == 0:
                    o_sb = opool.tile([P, 2, D], bf16, name="o_sb", tag="o")
                o_slice = o_sb[:, i % 2, :]
                if paths[i] == 1:
                    nc.vector.tensor_add(o_slice, cor[:], Xall[:, i, :])
                else:
                    t_sb = tpool.tile([P, D], bf16, name="t_sb", tag="t")
                    nc.scalar.copy(t_sb[:], cor[:])
                    if paths[i] == 2:
                        nc.vector.tensor_add(o_slice, t_sb[:], Xall[:, i, :])
                    else:
                        nc.gpsimd.tensor_add(o_slice, t_sb[:], Xall[:, i, :])
                if i % 2 == 1:
                    nc.sync.dma_start(
                        out[(i - 1) * P:(i + 1) * P, :].rearrange(
                            "(t p) d -> p t d", p=P
                        ),
                        o_sb[:],
                    )

    nc.finalize()
    return nc


def _run(inputs, trace=False, **kwargs):
    import ml_dtypes
    from concourse import bass_utils

    bf16 = ml_dtypes.bfloat16

    X = np.asarray(inputs["X"], dtype=np.float32)
    W = np.asarray(inputs["W"], dtype=np.float32)
    b = np.ascontiguousarray(np.asarray(inputs["b"], dtype=np.float32))

    with_bias = bool(np.any(b != 0.0))
    key = "nc_bias" if with_bias else "nc"
    if key not in _cache:
        _cache[key] = _build_nc(with_bias)
    nc = _cache[key]

    f8 = ml_dtypes.float8_e4m3

    Xb = np.ascontiguousarray(X.astype(bf16))
    Wtb = np.ascontiguousarray(W.T.astype(bf16))
    # fp8 X in row-pair interleaved layout: Xp8[s, p, j, d] = X[256s+128j+p, d]
    X8 = X.astype(f8).reshape(NCORES, NPAIR, 2, P, D).swapaxes(2, 3)

    in_maps = [
        {
            "X": Xb[i * NLOC:(i + 1) * NLOC],
            "Xp8": np.ascontiguousarray(X8[i]),
            "Wt": Wtb,
            "b": b,
        }
        for i in range(NCORES)
    ]
    res = bass_utils.run_bass_kernel_spmd(
        nc, in_maps, core_ids=list(range(NCORES)), trace=trace, **kwargs
    )
    outp = np.concatenate(
        [np.asarray(res.results[i]["out"]) for i in range(NCORES)], axis=0
    ).astype(np.float32)
    return outp, res


def kernel(**inputs):
    outp, _ = _run(inputs, trace=False)
    return outp
